# revision 6
# baseline (speedup 1.0000x reference)
"""Trainium2 Bass kernel for BatchedAdjacency (exact Gaussian-kernel MVM).

Math per batch b (n = H*W = 4096 pixels, d = 5 guide dims, L = 16 channels):
    W[i,j]   = exp(-0.5 * ||r_i - r_j||^2)
    out[l,i] = sum_j W[i,j] * s[j,l] - s[i,l]

Distribution: 8 cores = 4 batches x 2 row-halves (2048 output pixels each).

Device algorithm per core (ScalarE exp is the bottleneck: 8.39M exps at
1 elem/cycle/lane @ 1.2 GHz = 54.6 us pure stream):
  - MM1 computes -0.5*d2 tiles [j=128, i=512] directly via augmented features:
      lhsT rows = [r_hi(5), r_lo(5), r_hi(5), -0.5sq_hi, -0.5sq_lo, 1, 1]
      rhs  rows = [r_hi(5), r_hi(5), r_lo(5), 1, 1, -0.5sq_hi, -0.5sq_lo]
    so lhsT.T @ rhs = cross - 0.5 sq_j - 0.5 sq_i = -0.5 d2 (K = 19).
    j-blocks are packed into distinct 32-row PE strips (concurrent MMs):
    groups alternate QUAD (4 j-blocks -> 4-bank PSUM tile [128, 2048]) and
    TRIO (3 j-blocks -> 3-bank tile [128, 1536]), 9 groups per chunk --
    fewer, larger ACTIVATEs amortize the ~140-cycle per-instruction overhead.
  - ScalarE applies exp() PSUM->SBUF (fp16 out).
  - MM2 accumulates out[l, i] += s_rows[j, l]^T @ W[j, i] over all 32 j-blocks
    into a persistent PSUM bank; stationary is s fp16 (M = 16; the hi/lo
    split was dropped -- rel-err gate is 2e-2, this costs ~3e-4).  The single
    acc bank is partition-split: even chunks accumulate at partitions 0:16,
    odd chunks at 64:80 (col strip 2), so quad+trio+acc = 4+3+1 = 8 banks.
  - VectorE: out = acc - src (one op), then DMA to HBM.

Head optimizations: only the 19 real aug rows per 32-row strip are DMAed
(strided-partition APs, 76 of 128 packets), first-group slices prioritized
across the sync/scalar HWDGE rings + gpsimd SWDGE ring; PE warm-up fillers
sized to the DMA latency so HAM hits 8/8 right as the real stream starts.
"""

import sys

if "/opt/trn_rl_repo" not in sys.path:
    sys.path.insert(0, "/opt/trn_rl_repo")

import numpy as np

import concourse.bacc as bacc
import concourse.bass as bass
import concourse.mybir as mybir
import concourse.tile as tile
from concourse.bass_utils import run_bass_kernel_spmd


def install_ntff_hook() -> None:
    """Provide antenv.axon_hooks (absent in this image) so that
    run_bass_kernel_spmd can profile via the axon .so when tracing is
    requested -- and so a stray BASS_TRACE env var cannot crash the run."""
    import types

    if "antenv.axon_hooks" in sys.modules:
        return
    hook = None
    try:
        import antenv
        from trn_agent_boot.trn_boot import _ntff_profile_via_ctypes

        hook = _ntff_profile_via_ctypes("/opt/axon/libaxon_pjrt.so")
    except Exception:
        antenv = None
    mod = types.ModuleType("antenv.axon_hooks")
    mod._hook = hook
    mod.get_axon_ntff_profile_hook = lambda: mod._hook
    mod.set_axon_ntff_profile_hook = lambda h: setattr(mod, "_hook", h)
    sys.modules["antenv.axon_hooks"] = mod
    if antenv is not None:
        antenv.axon_hooks = mod


install_ntff_hook()

BS, L, D, H, W = 4, 16, 5, 64, 64
N = H * W            # 4096 pixels
NCORES = 8
HALF = N // 2        # output pixels per core
CHUNK = 512          # i-tile (PSUM bank / fp32 matmul free-dim limit)
NCHUNK = HALF // CHUNK
JB = 128             # j-block (contraction tile)
NJB = N // JB        # 32
KAUG = 19            # augmented feature count
# per-chunk groups: (first j-block, group size) alternating quad/trio
GROUPS = [(0, 4), (4, 3), (7, 4), (11, 3), (14, 4), (18, 3), (21, 4), (25, 3), (28, 4)]

FP16 = mybir.dt.float16
F32 = mybir.dt.float32

N_FILLER = 7         # PE warm-up matmuls bridging the input-DMA wait


R2 = 32 + KAUG      # rows 0:51 / 64:115 cover the live rows of a strip pair


def build_nc() -> bass.Bass:
    nc = bacc.Bacc()

    aug_j = nc.declare_dram_parameter("aug_j", [128, N], FP16, isOutput=False)
    aug_i = nc.declare_dram_parameter("aug_i", [128, HALF], FP16, isOutput=False)
    s_rows = nc.declare_dram_parameter("s_rows", [128, NJB * 16], FP16, isOutput=False)
    s_nat = nc.declare_dram_parameter("s_nat", [16, HALF], F32, isOutput=False)
    out = nc.declare_dram_parameter("out", [16, HALF], F32, isOutput=True)

    with tile.TileContext(nc) as tc:
        with (
            tc.tile_pool(name="const", bufs=1) as cpool,
            tc.tile_pool(name="wpool", bufs=3) as wpool,
            tc.tile_pool(name="pq", bufs=1, space="PSUM") as pqpool,
            tc.tile_pool(name="pt", bufs=1, space="PSUM") as ptpool,
            tc.tile_pool(name="apool", bufs=1, space="PSUM") as apool,
            tc.tile_pool(name="opool", bufs=2) as opool,
        ):
            aug_j_sb = cpool.tile([128, N], FP16)
            aug_i_sb = cpool.tile([128, HALF], FP16)
            s_rows_sb = cpool.tile([128, NJB * 16], FP16)
            s_nat_sb = cpool.tile([128, HALF], F32)

            # Input DMAs: only the live rows of each strip pair move (51 of
            # 64 partitions).  First-group slices (cols 0:512 of aug_j +
            # aug_i, all 4 strips) go first on the sync HWDGE + gpsimd SWDGE
            # rings so the first MM1 quad can start ~6 us in; the scalar
            # HWDGE ring takes the bulk so its queue stays clear for the
            # ACT table load + exp stream.
            nc.sync.dma_start(out=aug_j_sb[0:R2, :CHUNK], in_=aug_j[0:R2, :CHUNK])
            nc.gpsimd.dma_start(out=aug_i_sb[0:R2, :CHUNK], in_=aug_i[0:R2, :CHUNK])
            nc.sync.dma_start(
                out=aug_j_sb[64 : 64 + R2, :CHUNK], in_=aug_j[64 : 64 + R2, :CHUNK]
            )
            nc.gpsimd.dma_start(
                out=aug_i_sb[64 : 64 + R2, :CHUNK], in_=aug_i[64 : 64 + R2, :CHUNK]
            )
            nc.scalar.dma_start(out=aug_j_sb[0:R2, CHUNK:], in_=aug_j[0:R2, CHUNK:])
            nc.sync.dma_start(out=s_rows_sb[:, :256], in_=s_rows[:, :256])
            nc.scalar.dma_start(
                out=aug_j_sb[64 : 64 + R2, CHUNK:], in_=aug_j[64 : 64 + R2, CHUNK:]
            )
            nc.sync.dma_start(out=s_rows_sb[:, 256:], in_=s_rows[:, 256:])
            nc.scalar.dma_start(out=aug_i_sb[0:R2, CHUNK:], in_=aug_i[0:R2, CHUNK:])
            nc.scalar.dma_start(
                out=aug_i_sb[64 : 64 + R2, CHUNK:], in_=aug_i[64 : 64 + R2, CHUNK:]
            )
            # src rows 0:16 of s_nat land at partitions 0:16 AND 64:80 so the
            # DVE drain's lanes line up with either acc slice
            nc.gpsimd.dma_start(out=s_nat_sb[0:16, :], in_=s_nat[:, :])
            nc.gpsimd.dma_start(out=s_nat_sb[64:80, :], in_=s_nat[:, :])

            zt = cpool.tile([128, 640], FP16)
            nc.vector.memset(zt[:], 0.0)

            # PE warm-up: dependency-free zero matmuls run during the input
            # DMAs, sized so HAM reaches 8/8 right as real work starts.
            # They write the trio PSUM tile, which group 1 overwrites later.
            wz = ptpool.tile([128, 3 * CHUNK], F32, tag="pt", name="wz")
            for _ in range(N_FILLER):
                nc.tensor.matmul(
                    wz[:, :CHUNK],
                    lhsT=zt[:, :128],
                    rhs=zt[:, 128:640],
                    start=True,
                    stop=True,
                )

            # single acc bank, partition-split across alternate chunks
            acc = apool.tile([128, CHUNK], F32, tag="acc", name="acc")

            def emit_mm1(c, jb0, gs):
                isl = slice(c * CHUNK, (c + 1) * CHUNK)
                pool, tag = (pqpool, "pq") if gs == 4 else (ptpool, "pt")
                p = pool.tile([128, gs * CHUNK], F32, tag=tag, name=tag)
                for s in range(gs):
                    jb = jb0 + s
                    nc.tensor.matmul(
                        p[:, s * CHUNK : (s + 1) * CHUNK],
                        lhsT=aug_j_sb[32 * s : 32 * s + KAUG, jb * JB : (jb + 1) * JB],
                        rhs=aug_i_sb[32 * s : 32 * s + KAUG, isl],
                        start=True,
                        stop=True,
                        tile_position=(32 * s, 0),
                    )
                wt = wpool.tile([128, 4 * CHUNK], FP16, tag="w", name="wt")
                nc.scalar.activation(
                    wt[:, : gs * CHUNK],
                    p[:, : gs * CHUNK],
                    mybir.ActivationFunctionType.Exp,
                )
                return wt

            def emit_mm2(c, jb0, gs, wt):
                pbase = (c % 2) * 64
                ac = acc[pbase : pbase + 16, :]
                for s in range(gs):
                    jb = jb0 + s
                    nc.tensor.matmul(
                        ac,
                        lhsT=s_rows_sb[:, jb * 16 : (jb + 1) * 16],
                        rhs=wt[:, s * CHUNK : (s + 1) * CHUNK],
                        start=(jb == 0),
                        stop=(jb == NJB - 1),
                        skip_group_check=True,
                    )
                if jb0 + gs == NJB:  # chunk complete: drain + store
                    isl = slice(c * CHUNK, (c + 1) * CHUNK)
                    o = opool.tile([128, CHUNK], F32, tag="o", name="o")
                    osl = o[pbase : pbase + 16, :]
                    nc.vector.tensor_sub(osl, ac, s_nat_sb[pbase : pbase + 16, isl])
                    nc.sync.dma_start(out=out[:, isl], in_=osl)

            # software-pipelined emission, lag-2: the PE stream is
            # MM1(G), MM1(G+1), MM2(G), MM1(G+2), MM2(G+1), ... so the PE
            # never stalls on the exp of the current group before starting
            # the next group's MM1s.
            pend = []
            for c in range(NCHUNK):
                for jb0, gs in GROUPS:
                    wt = emit_mm1(c, jb0, gs)
                    if len(pend) == 2:
                        emit_mm2(*pend.pop(0))
                    pend.append((c, jb0, gs, wt))
            for args in pend:
                emit_mm2(*args)

    nc.finalize()
    return nc


def _hi_lo(x: np.ndarray):
    hi = x.astype(np.float16)
    lo = (x - hi.astype(np.float32)).astype(np.float16)
    return hi, lo


def prep_core_inputs(src: np.ndarray, guide: np.ndarray) -> list[dict]:
    """Shard full inputs into the 8 per-core input maps (host-side layout prep)."""
    in_maps = []
    for b in range(BS):
        refs = np.ascontiguousarray(guide[b].reshape(D, N), dtype=np.float32)
        srcs = np.ascontiguousarray(src[b].reshape(L, N), dtype=np.float32)
        sq = (refs.astype(np.float64) ** 2).sum(0)
        r_hi, r_lo = _hi_lo(refs)
        q_hi, q_lo = _hi_lo((-0.5 * sq).astype(np.float32))
        ones = np.ones((1, N), np.float16)
        augj = np.concatenate(
            [r_hi, r_lo, r_hi, q_hi[None], q_lo[None], ones, ones], axis=0
        )
        augi = np.concatenate(
            [r_hi, r_hi, r_lo, ones, ones, q_hi[None], q_lo[None]], axis=0
        )
        augj_rep = np.zeros((128, N), np.float16)
        augi_rep = np.zeros((128, N), np.float16)
        for s in range(4):
            augj_rep[32 * s : 32 * s + KAUG] = augj
            augi_rep[32 * s : 32 * s + KAUG] = augi
        s16 = srcs.astype(np.float16)
        s_rows = np.zeros((128, NJB * 16), np.float16)
        for jb in range(NJB):
            s_rows[:, 16 * jb : 16 * jb + 16] = s16[:, jb * JB : (jb + 1) * JB].T
        for h in range(2):
            isl = slice(h * HALF, (h + 1) * HALF)
            in_maps.append(
                {
                    "aug_j": augj_rep,
                    "aug_i": np.ascontiguousarray(augi_rep[:, isl]),
                    "s_rows": s_rows,
                    "s_nat": np.ascontiguousarray(srcs[:, isl]),
                }
            )
    return in_maps


_NC_CACHE = None


def _get_nc() -> bass.Bass:
    global _NC_CACHE
    if _NC_CACHE is None:
        _NC_CACHE = build_nc()
    return _NC_CACHE


def run_on_hw(in_maps, **kwargs):
    return run_bass_kernel_spmd(_get_nc(), in_maps, core_ids=list(range(NCORES)), **kwargs)


def assemble_output(results: list[dict]) -> np.ndarray:
    out = np.empty((BS, L, N), np.float32)
    for b in range(BS):
        for h in range(2):
            out[b, :, h * HALF : (h + 1) * HALF] = results[2 * b + h]["out"]
    return out.reshape(BS, L, H, W)


def kernel(src_imgs: np.ndarray, guide_imgs: np.ndarray) -> np.ndarray:
    src = np.asarray(src_imgs, dtype=np.float32)
    guide = np.asarray(guide_imgs, dtype=np.float32)
    in_maps = prep_core_inputs(src, guide)
    res = run_on_hw(in_maps)
    return assemble_output(res.results)


# revision 11
# speedup vs baseline: 1.0796x; 1.0796x over previous
"""Trainium2 Bass kernel for BatchedAdjacency (exact Gaussian-kernel MVM).

Math per batch b (n = H*W = 4096 pixels, d = 5 guide dims, L = 16 channels):
    W[i,j]   = exp(-0.5 * ||r_i - r_j||^2)
    out[l,i] = sum_j W[i,j] * s[j,l] - s[i,l]

Distribution: 8 cores = 4 batches x 2 row-halves (2048 output pixels each).

Device algorithm per core (ScalarE exp is the bottleneck: 8.39M exps at
1 elem/cycle/lane @ 1.2 GHz = 54.6 us pure stream):
  - MM1 computes -0.5*d2 tiles [j=128, i=512] directly via augmented features:
      lhsT rows = [r_hi(5), r_lo(5), r_hi(5), -0.5sq_hi, -0.5sq_lo, 1, 1]
      rhs  rows = [r_hi(5), r_hi(5), r_lo(5), 1, 1, -0.5sq_hi, -0.5sq_lo]
    so lhsT.T @ rhs = cross - 0.5 sq_j - 0.5 sq_i = -0.5 d2 (K = 19).
    j-blocks are packed into distinct 32-row PE strips (concurrent MMs):
    groups alternate QUAD (4 j-blocks -> 4-bank PSUM tile [128, 2048]) and
    TRIO (3 j-blocks -> 3-bank tile [128, 1536]), 9 groups per chunk --
    fewer, larger ACTIVATEs amortize the ~140-cycle per-instruction overhead.
  - ScalarE applies exp() PSUM->SBUF (fp16 out).
  - MM2 accumulates out[l, i] += s_rows[j, l]^T @ W[j, i] over all 32 j-blocks
    into a persistent PSUM bank; stationary is s fp16 (M = 16; the hi/lo
    split was dropped -- rel-err gate is 2e-2, this costs ~3e-4).  The single
    acc bank is partition-split: even chunks accumulate at partitions 0:16,
    odd chunks at 64:80 (col strip 2), so quad+trio+acc = 4+3+1 = 8 banks.
  - VectorE: out = acc - src (one op), then DMA to HBM.

Head optimizations: only the 19 real aug rows per 32-row strip are DMAed
(strided-partition APs, 76 of 128 packets), first-group slices prioritized
across the sync/scalar HWDGE rings + gpsimd SWDGE ring; PE warm-up fillers
sized to the DMA latency so HAM hits 8/8 right as the real stream starts.
"""

import sys

if "/opt/trn_rl_repo" not in sys.path:
    sys.path.insert(0, "/opt/trn_rl_repo")

import numpy as np

import concourse.bacc as bacc
import concourse.bass as bass
import concourse.mybir as mybir
import concourse.tile as tile
from concourse.bass_utils import run_bass_kernel_spmd


def install_ntff_hook() -> None:
    """Provide antenv.axon_hooks (absent in this image) so that
    run_bass_kernel_spmd can profile via the axon .so when tracing is
    requested -- and so a stray BASS_TRACE env var cannot crash the run."""
    import types

    if "antenv.axon_hooks" in sys.modules:
        return
    hook = None
    try:
        import antenv
        from trn_agent_boot.trn_boot import _ntff_profile_via_ctypes

        hook = _ntff_profile_via_ctypes("/opt/axon/libaxon_pjrt.so")
    except Exception:
        antenv = None
    mod = types.ModuleType("antenv.axon_hooks")
    mod._hook = hook
    mod.get_axon_ntff_profile_hook = lambda: mod._hook
    mod.set_axon_ntff_profile_hook = lambda h: setattr(mod, "_hook", h)
    sys.modules["antenv.axon_hooks"] = mod
    if antenv is not None:
        antenv.axon_hooks = mod


install_ntff_hook()

BS, L, D, H, W = 4, 16, 5, 64, 64
N = H * W            # 4096 pixels
NCORES = 8
HALF = N // 2        # output pixels per core
CHUNK = 512          # i-tile (PSUM bank / fp32 matmul free-dim limit)
NCHUNK = HALF // CHUNK
JB = 128             # j-block (contraction tile)
NJB = N // JB        # 32
KAUG = 19            # augmented feature count
GS = 3               # j-blocks per PSUM trio tile
# per-chunk groups: (first j-block, group size): 10 trios + final duo
GROUPS = [(g, min(GS, NJB - g)) for g in range(0, NJB, GS)]

FP16 = mybir.dt.float16
F32 = mybir.dt.float32

N_FILLER = 6         # PE warm-up matmuls bridging the input-DMA wait


R2 = 32 + KAUG      # rows 0:51 / 64:115 cover the live rows of a strip pair


def build_nc() -> bass.Bass:
    nc = bacc.Bacc()

    aug_j = nc.declare_dram_parameter("aug_j", [128, N], FP16, isOutput=False)
    aug_i = nc.declare_dram_parameter("aug_i", [128, HALF], FP16, isOutput=False)
    s_rows = nc.declare_dram_parameter("s_rows", [128, NJB * 16], FP16, isOutput=False)
    s_nat = nc.declare_dram_parameter("s_nat", [16, HALF], F32, isOutput=False)
    out = nc.declare_dram_parameter("out", [16, HALF], F32, isOutput=True)

    with tile.TileContext(nc) as tc:
        with (
            tc.tile_pool(name="const", bufs=1) as cpool,
            tc.tile_pool(name="wpool", bufs=3) as wpool,
            tc.tile_pool(name="ppool", bufs=2, space="PSUM") as ppool,
            tc.tile_pool(name="apool", bufs=1, space="PSUM") as apool,
            tc.tile_pool(name="opool", bufs=2) as opool,
        ):
            aug_j_sb = cpool.tile([128, N], FP16)
            aug_i_sb = cpool.tile([128, HALF], FP16)
            s_rows_sb = cpool.tile([128, NJB * 16], FP16)
            s_nat_sb = cpool.tile([128, HALF], F32)

            # Input DMAs: only the live rows of each strip pair move (51 of
            # 64 partitions).  The two HWDGE rings carry the latency-critical
            # pieces -- strip pair {0,1} on sync, {2,3} on scalar (whose ring
            # starts ~1.3us later behind the hoisted ACT table load) -- with
            # aug_j column pieces sized to land just ahead of the trio that
            # consumes them.  The laggy gpsimd SWDGE ring gets only
            # late-needed data (aug_i bulk for chunks 1+, s_nat).
            lo, hi = slice(0, R2), slice(64, 64 + R2)
            nc.sync.dma_start(out=aug_j_sb[lo, :384], in_=aug_j[lo, :384])
            nc.scalar.dma_start(out=aug_j_sb[hi, :384], in_=aug_j[hi, :384])
            nc.sync.dma_start(out=aug_i_sb[lo, :CHUNK], in_=aug_i[lo, :CHUNK])
            nc.scalar.dma_start(out=aug_i_sb[hi, :CHUNK], in_=aug_i[hi, :CHUNK])
            nc.sync.dma_start(out=s_rows_sb[:, :256], in_=s_rows[:, :256])
            nc.sync.dma_start(out=aug_j_sb[lo, 384:1920], in_=aug_j[lo, 384:1920])
            nc.scalar.dma_start(out=aug_j_sb[hi, 384:1920], in_=aug_j[hi, 384:1920])
            nc.sync.dma_start(out=s_rows_sb[:, 256:], in_=s_rows[:, 256:])
            nc.sync.dma_start(out=aug_j_sb[lo, 1920:3072], in_=aug_j[lo, 1920:3072])
            nc.scalar.dma_start(out=aug_j_sb[hi, 1920:3072], in_=aug_j[hi, 1920:3072])
            nc.sync.dma_start(out=aug_j_sb[lo, 3072:], in_=aug_j[lo, 3072:])
            nc.scalar.dma_start(out=aug_j_sb[hi, 3072:], in_=aug_j[hi, 3072:])
            nc.sync.dma_start(out=aug_i_sb[lo, CHUNK:], in_=aug_i[lo, CHUNK:])
            nc.scalar.dma_start(out=aug_i_sb[hi, CHUNK:], in_=aug_i[hi, CHUNK:])
            # src rows 0:16 of s_nat land at partitions 0:16 AND 64:80 so the
            # DVE drain's lanes line up with either acc slice
            nc.gpsimd.dma_start(out=s_nat_sb[0:16, :], in_=s_nat[:, :])
            nc.gpsimd.dma_start(out=s_nat_sb[64:80, :], in_=s_nat[:, :])

            zt = cpool.tile([128, 640], FP16)
            nc.vector.memset(zt[:], 0.0)

            # PE warm-up: dependency-free zero matmuls run during the input
            # DMAs, sized so HAM reaches 8/8 right as real work starts.
            # They write one trio PSUM buffer, which group 1 reuses later.
            wz = ppool.tile([128, GS * CHUNK], F32, tag="p", name="wz")
            for _ in range(N_FILLER):
                nc.tensor.matmul(
                    wz[:, :CHUNK],
                    lhsT=zt[:, :128],
                    rhs=zt[:, 128:640],
                    start=True,
                    stop=True,
                )

            # single acc bank, partition-split across alternate chunks
            acc = apool.tile([128, CHUNK], F32, tag="acc", name="acc")

            def emit_mm1(c, jb0, gs):
                isl = slice(c * CHUNK, (c + 1) * CHUNK)
                p = ppool.tile([128, GS * CHUNK], F32, tag="p", name="p")
                for s in range(gs):
                    jb = jb0 + s
                    nc.tensor.matmul(
                        p[:, s * CHUNK : (s + 1) * CHUNK],
                        lhsT=aug_j_sb[32 * s : 32 * s + KAUG, jb * JB : (jb + 1) * JB],
                        rhs=aug_i_sb[32 * s : 32 * s + KAUG, isl],
                        start=True,
                        stop=True,
                        tile_position=(32 * s, 0),
                    )
                wt = wpool.tile([128, GS * CHUNK], FP16, tag="w", name="wt")
                nc.scalar.activation(
                    wt[:, : gs * CHUNK],
                    p[:, : gs * CHUNK],
                    mybir.ActivationFunctionType.Exp,
                )
                return wt

            def emit_mm2(c, jb0, gs, wt):
                pbase = (c % 2) * 64
                ac = acc[pbase : pbase + 16, :]
                for s in range(gs):
                    jb = jb0 + s
                    nc.tensor.matmul(
                        ac,
                        lhsT=s_rows_sb[:, jb * 16 : (jb + 1) * 16],
                        rhs=wt[:, s * CHUNK : (s + 1) * CHUNK],
                        start=(jb == 0),
                        stop=(jb == NJB - 1),
                        skip_group_check=True,
                    )
                if jb0 + gs == NJB:  # chunk complete: drain + store
                    isl = slice(c * CHUNK, (c + 1) * CHUNK)
                    o = opool.tile([128, CHUNK], F32, tag="o", name="o")
                    osl = o[pbase : pbase + 16, :]
                    nc.vector.tensor_sub(osl, ac, s_nat_sb[pbase : pbase + 16, isl])
                    nc.sync.dma_start(out=out[:, isl], in_=osl)

            # software-pipelined emission, lag-2: the PE stream is
            # MM1(G), MM1(G+1), MM2(G), MM1(G+2), MM2(G+1), ... so the PE
            # never stalls on the exp of the current group before starting
            # the next group's MM1s.  The last chunk drops to lag-1 so only
            # the final duo's MM2s remain after the last exp.
            pend = []
            for c in range(NCHUNK):
                lag = 1 if c == NCHUNK - 1 else 2
                for jb0, gs in GROUPS:
                    wt = emit_mm1(c, jb0, gs)
                    pend.append((c, jb0, gs, wt))
                    while len(pend) > lag:
                        emit_mm2(*pend.pop(0))
            for args in pend:
                emit_mm2(*args)

    nc.finalize()
    return nc


def _hi_lo(x: np.ndarray):
    hi = x.astype(np.float16)
    lo = (x - hi.astype(np.float32)).astype(np.float16)
    return hi, lo


def prep_core_inputs(src: np.ndarray, guide: np.ndarray) -> list[dict]:
    """Shard full inputs into the 8 per-core input maps (host-side layout prep)."""
    in_maps = []
    for b in range(BS):
        refs = np.ascontiguousarray(guide[b].reshape(D, N), dtype=np.float32)
        srcs = np.ascontiguousarray(src[b].reshape(L, N), dtype=np.float32)
        sq = (refs.astype(np.float64) ** 2).sum(0)
        r_hi, r_lo = _hi_lo(refs)
        q_hi, q_lo = _hi_lo((-0.5 * sq).astype(np.float32))
        ones = np.ones((1, N), np.float16)
        augj = np.concatenate(
            [r_hi, r_lo, r_hi, q_hi[None], q_lo[None], ones, ones], axis=0
        )
        augi = np.concatenate(
            [r_hi, r_hi, r_lo, ones, ones, q_hi[None], q_lo[None]], axis=0
        )
        augj_rep = np.zeros((128, N), np.float16)
        augi_rep = np.zeros((128, N), np.float16)
        for s in range(4):
            augj_rep[32 * s : 32 * s + KAUG] = augj
            augi_rep[32 * s : 32 * s + KAUG] = augi
        s16 = srcs.astype(np.float16)
        s_rows = np.zeros((128, NJB * 16), np.float16)
        for jb in range(NJB):
            s_rows[:, 16 * jb : 16 * jb + 16] = s16[:, jb * JB : (jb + 1) * JB].T
        for h in range(2):
            isl = slice(h * HALF, (h + 1) * HALF)
            in_maps.append(
                {
                    "aug_j": augj_rep,
                    "aug_i": np.ascontiguousarray(augi_rep[:, isl]),
                    "s_rows": s_rows,
                    "s_nat": np.ascontiguousarray(srcs[:, isl]),
                }
            )
    return in_maps


_NC_CACHE = None


def _get_nc() -> bass.Bass:
    global _NC_CACHE
    if _NC_CACHE is None:
        _NC_CACHE = build_nc()
    return _NC_CACHE


def run_on_hw(in_maps, **kwargs):
    return run_bass_kernel_spmd(_get_nc(), in_maps, core_ids=list(range(NCORES)), **kwargs)


def assemble_output(results: list[dict]) -> np.ndarray:
    out = np.empty((BS, L, N), np.float32)
    for b in range(BS):
        for h in range(2):
            out[b, :, h * HALF : (h + 1) * HALF] = results[2 * b + h]["out"]
    return out.reshape(BS, L, H, W)


def kernel(src_imgs: np.ndarray, guide_imgs: np.ndarray) -> np.ndarray:
    src = np.asarray(src_imgs, dtype=np.float32)
    guide = np.asarray(guide_imgs, dtype=np.float32)
    in_maps = prep_core_inputs(src, guide)
    res = run_on_hw(in_maps)
    return assemble_output(res.results)
